# revision 19
# baseline (speedup 1.0000x reference)
"""Trainium2 Bass kernel for a GQA attention block (B=8,T=1024,C=1024,H=16,HKV=4).

One batch element per NeuronCore (8 cores). Per core:
  q = x@w_q.T ; kv = x@w_kv.T ; QK-RMSNorm ; RoPE ; GQA attention with
  soft logit cap 50*tanh(s/50), causal softmax ; y = att_out @ w_c.T.

Key design:
  - All matmuls in float32r (full PE rate at N>=256), fp32 PSUM accumulate.
  - Projections in transposed layout [o, t]: per-head tiles are [HD, T] =
    exactly the lhsT/rhs layout QK^T needs. v in natural [t, o] layout with a
    ones column appended so att@V emits softmax denominators for free.
  - Soft cap => no softmax max-subtraction needed (|logit| <= 50).
  - rstd = exp(-0.5*ln(ms/HD + eps)) on ACT (Rsqrt table is banned);
    partition broadcasts on idle GPSIMD; RMSNorm weights folded into
    host-precomputed RoPE tables; rstd folded into q/k prescale.
  - Scores transposed (s^T [kt, qt]) so p^T feeds att@V directly; causal via
    partial-N matmuls + one constant [128,128] upper-triangular mask on
    diagonal blocks; junk in never-streamed PSUM regions is bounded by tanh
    and never read by att@V.
"""

import sys

sys.path.insert(0, "/opt/trn_rl_repo")

import numpy as np

import concourse.bass as bass  # noqa: F401
import concourse.mybir as mybir
from concourse import bacc
from concourse import tile
from concourse.bass_utils import run_bass_kernel_spmd

F32 = mybir.dt.float32
F32R = mybir.dt.float32r
AF = mybir.ActivationFunctionType

B, T, C = 8, 1024, 1024
H, HKV, HD = 16, 4, 64
G = H // HKV          # 4
CAP = 50.0
THETA = 10000.0
EPS = 1e-6
NCH = C // 128        # 8 contraction chunks
QCH = 8               # q output chunks (2 heads each)
KCH = 2               # k output chunks
TT = T // 128         # 8 t subtiles
HHD = HD // 2         # 32


def r(ap):
    """float32r view of an fp32 AP (for matmul inputs)."""
    return ap.bitcast(F32R)


def _build():
    nc = bacc.Bacc("TRN2", target_bir_lowering=False, debug=True)

    xT = nc.dram_tensor("xT", [C, T], F32R, kind="ExternalInput")
    wqT = nc.dram_tensor("wqT", [C, C], F32R, kind="ExternalInput")
    wkvT = nc.dram_tensor("wkvT", [C, 512], F32R, kind="ExternalInput")
    wcT = nc.dram_tensor("wcT", [C, C], F32R, kind="ExternalInput")
    qrope = nc.dram_tensor("qrope", [128, T], F32, kind="ExternalInput")
    krope = nc.dram_tensor("krope", [128, T], F32, kind="ExternalInput")
    trim = nc.dram_tensor("trim", [128, 128], F32, kind="ExternalInput")
    indq = nc.dram_tensor("indq", [128, 8], F32R, kind="ExternalInput")
    indh = nc.dram_tensor("indh", [4, 256], F32R, kind="ExternalInput")
    ones32 = nc.dram_tensor("ones32", [128, 32], F32R, kind="ExternalInput")
    out = nc.dram_tensor("out", [T, C], F32, kind="ExternalOutput")

    with tile.TileContext(nc) as tc:
        with (
            tc.tile_pool(name="const", bufs=1) as const,
            tc.tile_pool(name="big", bufs=1) as big,
            tc.tile_pool(name="wq_pool", bufs=8) as wq_pool,
            tc.tile_pool(name="wc_pool", bufs=8) as wc_pool,
            tc.tile_pool(name="work", bufs=2) as work,
            tc.tile_pool(name="attn", bufs=1) as attn,
            tc.tile_pool(name="psum", bufs=1, space="PSUM") as psum,
        ):
            # ---------------- constants ----------------
            zeros_c = const.tile([128, 1], F32)
            nc.vector.memset(zeros_c, 0.0)
            eps_c = const.tile([128, 1], F32)
            nc.vector.memset(eps_c, EPS)
            nc.const_aps.aps[(F32, 0.0)] = zeros_c
            nc.const_aps.aps[(F32, EPS)] = eps_c

            qrope_sb = const.tile([128, T], F32)
            nc.sync.dma_start(qrope_sb, qrope[:])
            krope_sb = const.tile([128, T], F32)
            nc.sync.dma_start(krope_sb, krope[:])
            tri_sb = const.tile([128, 128], F32)
            nc.sync.dma_start(tri_sb, trim[:])
            indq_sb = const.tile([128, 8], F32R)
            nc.sync.dma_start(indq_sb, indq[:])
            indh_sb = const.tile([4, 256], F32R)
            nc.sync.dma_start(indh_sb, indh[:])

            # ---------------- resident activations ----------------
            xsb = big.tile([128, NCH * T], F32R, tag="xy")  # x^T chunks
            for cc in range(NCH):
                nc.sync.dma_start(xsb[:, cc * T:(cc + 1) * T],
                                  xT[cc * 128:(cc + 1) * 128, :])
            qhat = big.tile([128, QCH * T], F32R, tag="qhat")
            khat = big.tile([128, KCH * T], F32R, tag="khat")
            # partition-swapped copy of khat (PE needs lhsT/rhs at same base)
            khat_sw = big.tile([128, KCH * T], F32R, tag="khat_sw")
            vhat = big.tile([128, TT * (HKV * 65)], F32R, tag="vhat")
            # ones columns (one per (tch, kv-head)) via a single strided DMA
            nc.sync.dma_start(vhat[:, 64:TT * (HKV * 65):65], ones32[:])

            # kv weights: [128, 512] x 8 chunks, in wc_pool's slots (wc loads
            # happen after v-proj is done, so the slots rotate naturally).
            wkv_tiles = []
            for cc in range(NCH):
                wkv_t = wc_pool.tile([128, 512], F32R, tag="wc", name=f"wkv{cc}")
                nc.sync.dma_start(wkv_t, wkvT[cc * 128:(cc + 1) * 128, :])
                wkv_tiles.append(wkv_t)

            # ---------------- transposed projection (+sumsq+RoPE+rstd) ------
            def proj_T(och_total, get_w, rope_sb, hat):
                """och pairs form rstd groups of 4 heads each."""
                mq = {}
                for och in range(och_total):
                    g = och // 2
                    for th in range(2):
                        ps = psum.tile([128, 512], F32, tag="proj", bufs=2,
                                       name=f"pp{och}_{th}")
                        for cc in range(NCH):
                            nc.tensor.matmul(
                                ps,
                                lhsT=r(get_w(cc, och)),
                                rhs=r(xsb[:, cc * T + th * 512:cc * T + (th + 1) * 512]),
                                start=(cc == 0), stop=(cc == NCH - 1),
                            )
                        # raw sumsq over head dims (accumulate over och pair);
                        # ACT Square (one PSUM read; DVE would need two)
                        q2t = work.tile([128, 512], F32R, tag="q2", bufs=2)
                        nc.scalar.square(q2t, ps)
                        if (g, th) not in mq:
                            # shares PSUM slots with the (later-phase) av tiles
                            mq[(g, th)] = psum.tile([4, 512], F32, tag="av",
                                                    bufs=2, name=f"mq{g}_{th}")
                        ind = indq_sb[:, 0:4] if och % 2 == 0 else indq_sb[:, 4:8]
                        nc.tensor.matmul(mq[(g, th)], lhsT=r(ind), rhs=r(q2t),
                                         start=(och % 2 == 0), stop=(och % 2 == 1))
                        # RoPE (norm weights folded into tables): psum -> hat
                        hb = och * T + th * 512
                        rs = slice(th * 512, (th + 1) * 512)
                        for hh in range(2):
                            pb = hh * 64
                            ta = work.tile([32, 512], F32, tag="ropea", bufs=3)
                            tb = work.tile([32, 512], F32, tag="ropeb", bufs=3)
                            nc.vector.tensor_mul(ta, ps[pb:pb + 32, :],
                                                 rope_sb[0:32, rs])
                            nc.vector.tensor_mul(tb, ps[pb + 32:pb + 64, :],
                                                 rope_sb[32:64, rs])
                            nc.vector.tensor_add(hat[pb:pb + 32, hb:hb + 512],
                                                 ta, tb)
                            ta2 = work.tile([32, 512], F32, tag="ropea", bufs=3)
                            tb2 = work.tile([32, 512], F32, tag="ropeb", bufs=3)
                            nc.vector.tensor_mul(ta2, ps[pb + 32:pb + 64, :],
                                                 rope_sb[96:128, rs])
                            nc.vector.tensor_mul(tb2, ps[pb:pb + 32, :],
                                                 rope_sb[64:96, rs])
                            nc.vector.tensor_sub(hat[pb + 32:pb + 64, hb:hb + 512],
                                                 ta2, tb2)
                    if och % 2 == 1:
                        # rstd for heads 4g..4g+3, then prescale hat rows
                        msq_t = work.tile([4, T], F32, tag="msq_sb", bufs=2)
                        for th in range(2):
                            nc.vector.tensor_copy(
                                msq_t[:, th * 512:(th + 1) * 512], mq[(g, th)])
                        lnt = work.tile([4, T], F32, tag="lnt", bufs=2)
                        nc.scalar.activation(lnt, msq_t, AF.Ln,
                                             bias=EPS, scale=1.0 / HD)
                        rstd_t = work.tile([4, T], F32R, tag="rstd", bufs=2)
                        nc.scalar.activation(rstd_t, lnt, AF.Exp, scale=-0.5)
                        # broadcast rstd across head rows via indicator matmul,
                        # then prescale hat in one [128,512] mul per (och, th)
                        for oo in range(2):
                            oc = g * 2 + oo
                            ind2 = indh_sb[:, oo * 128:(oo + 1) * 128]
                            for th in range(2):
                                bc = psum.tile([128, 512], F32, tag="proj",
                                               bufs=2, name=f"bc{oc}_{th}")
                                nc.tensor.matmul(
                                    bc, lhsT=r(ind2),
                                    rhs=r(rstd_t[:, th * 512:(th + 1) * 512]),
                                    start=True, stop=True)
                                sl = slice(oc * T + th * 512,
                                           oc * T + (th + 1) * 512)
                                nc.vector.tensor_mul(hat[:, sl], hat[:, sl], bc)

            # k projection first (unblocks attention early)
            proj_T(KCH,
                   lambda cc, och: wkv_tiles[cc][:, och * 128:(och + 1) * 128],
                   krope_sb, khat)

            # swapped-half copy of khat for base-partition matching
            for koch in range(KCH):
                sl = slice(koch * T, (koch + 1) * T)
                nc.vector.tensor_copy(khat_sw[0:64, sl], khat[64:128, sl])
                nc.vector.tensor_copy(khat_sw[64:128, sl], khat[0:64, sl])

            # q projection: wq streamed as [128,128] tiles, 8 live per och
            def get_wq(cc, och):
                t_ = wq_pool.tile([128, 128], F32R, tag="wq", name=f"wq{och}_{cc}")
                nc.sync.dma_start(
                    t_, wqT[cc * 128:(cc + 1) * 128, och * 128:(och + 1) * 128])
                return t_

            proj_T(QCH, get_wq, qrope_sb, qhat)

            # v projection LAST among projections (xsb's final reader, so the
            # xsb->yatt slot handoff doesn't stall the attention pipeline)
            for tch in range(TT):
                ps = psum.tile([128, 256], F32, tag="proj", bufs=2,
                               name=f"vps{tch}")
                for cc in range(NCH):
                    nc.tensor.matmul(
                        ps,
                        lhsT=r(xsb[:, cc * T + tch * 128:cc * T + (tch + 1) * 128]),
                        rhs=r(wkv_tiles[cc][:, 256:512]),
                        start=(cc == 0), stop=(cc == NCH - 1),
                    )
                vb = tch * (HKV * 65)
                for n in range(HKV):
                    nc.vector.tensor_copy(vhat[:, vb + n * 65:vb + n * 65 + 64],
                                          ps[:, n * 64:(n + 1) * 64])

            # ---------------- attention ----------------
            yatt = big.tile([128, QCH * T], F32R, tag="xy", name="yatt")
            STACKS = [(0, 0, 0), (1, 0, 1), (1, 4, 1)]  # (qi, j_lo, pT_idx)

            for h in range(H):
                och, hh = h // 2, h % 2
                n = h // G
                koch, khh = n // 2, n % 2
                qrow = qhat[hh * 64:(hh + 1) * 64, och * T:(och + 1) * T]
                ksrc = khat if khh == hh else khat_sw
                krow = ksrc[hh * 64:(hh + 1) * 64, koch * T:(koch + 1) * T]

                pT0 = attn.tile([128, 2048], F32R, tag="pT0", name=f"pT0_{h}")
                pT1 = attn.tile([128, 4096], F32R, tag="pT1", name=f"pT1_{h}")
                pts = [pT0, pT1]

                for qi, jlo, pti in STACKS:
                    st = psum.tile([128, 2048], F32, tag="stack", bufs=1,
                                   name=f"st{h}_{qi}_{jlo}")
                    for jj in range(4):
                        j = jlo + jj
                        # full N=512 (even below-diagonal): keeps PSUM fully
                        # written; att@V's partial-N never streams the junk
                        nc.tensor.matmul(
                            st[:, jj * 512:(jj + 1) * 512],
                            lhsT=r(krow[:, j * 128:(j + 1) * 128]),
                            rhs=r(qrow[:, qi * 512:(qi + 1) * 512]),
                            start=True, stop=True,
                        )
                    pcol = (jlo // 4) * 2048 if pti == 1 else 0
                    nc.scalar.activation(pts[pti][:, pcol:pcol + 2048], st,
                                         AF.Tanh, scale=1.0 / (8.0 * CAP))
                # exp(50*t) in place
                nc.scalar.activation(pT0, pT0, AF.Exp, scale=CAP)
                nc.scalar.activation(pT1, pT1, AF.Exp, scale=CAP)
                # diagonal masks + junk zeroing on straddle stacks
                for pt, base in ((pT0, 0), (pT1, 2048)):
                    for jj in range(4):
                        col = base + jj * 512 + jj * 128
                        nc.vector.tensor_mul(pt[:, col:col + 128],
                                             pt[:, col:col + 128], tri_sb)
                    # zero the causally-dead [256:384) of block jj=3 (memset
                    # doesn't codegen for f32r; values are finite so *0 works)
                    zsl = pt[:, base + 3 * 512 + 256: base + 3 * 512 + 384]
                    nc.vector.tensor_scalar_mul(zsl, zsl, 0.0)

                # att@V (+ denominator row via ones column)
                for qi in range(2):
                    av = psum.tile([65, 512], F32, tag="av", bufs=2,
                                   name=f"av{h}_{qi}")
                    pt = pts[qi]
                    jhi = 4 * (qi + 1)
                    for j in range(jhi):
                        rr_ = j - 4 * qi
                        off = 0 if rr_ < 0 else min(128 * rr_, 256)
                        nc.tensor.matmul(
                            av[:, off:512],
                            lhsT=r(vhat[:, j * (HKV * 65) + n * 65:
                                        j * (HKV * 65) + (n + 1) * 65]),
                            rhs=r(pt[:, j * 512 + off:(j + 1) * 512]),
                            start=(j == 0), stop=(j == jhi - 1),
                        )
                    rr2 = work.tile([1, 512], F32, tag="rr", bufs=2)
                    nc.vector.reciprocal(rr2, av[64:65, :])
                    rb2 = work.tile([64, 512], F32, tag="rb", bufs=2)
                    nc.gpsimd.partition_broadcast(rb2, rr2, channels=64)
                    nc.vector.tensor_mul(
                        yatt[hh * 64:(hh + 1) * 64,
                             och * T + qi * 512:och * T + (qi + 1) * 512],
                        av[0:64, :], rb2)

            # ---------------- c_proj ----------------
            for oh in range(2):
                wc_tiles = []
                for cc in range(NCH):
                    wc_t = wc_pool.tile([128, 512], F32R, tag="wc",
                                        name=f"wc{oh}_{cc}")
                    nc.sync.dma_start(
                        wc_t, wcT[cc * 128:(cc + 1) * 128,
                                  oh * 512:(oh + 1) * 512])
                    wc_tiles.append(wc_t)
                for tch in range(TT):
                    ps = psum.tile([128, 512], F32, tag="proj", bufs=2,
                                   name=f"cp{oh}_{tch}")
                    for cc in range(NCH):
                        nc.tensor.matmul(
                            ps,
                            lhsT=r(yatt[:, cc * T + tch * 128:
                                        cc * T + (tch + 1) * 128]),
                            rhs=r(wc_tiles[cc]),
                            start=(cc == 0), stop=(cc == NCH - 1),
                        )
                    osb = work.tile([128, 512], F32, tag="osb", bufs=2)
                    nc.vector.tensor_copy(osb, ps)
                    nc.sync.dma_start(
                        out[tch * 128:(tch + 1) * 128, oh * 512:(oh + 1) * 512],
                        osb)

    nc.compile()
    return nc


_NC_CACHE = None


def _get_nc():
    global _NC_CACHE
    if _NC_CACHE is None:
        _NC_CACHE = _build()
    return _NC_CACHE


def _host_prep(x, w_q, w_kv, w_c, q_norm_w, k_norm_w):
    f = np.float32
    xT = np.ascontiguousarray(np.transpose(np.asarray(x), (0, 2, 1))).astype(f, copy=False)
    wqT = np.ascontiguousarray(np.asarray(w_q).T).astype(f, copy=False)
    wkvT = np.ascontiguousarray(np.asarray(w_kv).T).astype(f, copy=False)
    wcT = np.ascontiguousarray(np.asarray(w_c).T).astype(f, copy=False)

    inv_freq = 1.0 / (THETA ** (np.arange(0, HD, 2, dtype=np.float32) / HD))
    pos = np.arange(T, dtype=np.float32)
    freqs = np.outer(pos, inv_freq)            # [T, 32]
    cosT = np.cos(freqs).T.astype(f)           # [32, T]
    sinT = np.sin(freqs).T.astype(f)

    def rope_pack(w):
        w1 = np.asarray(w)[:HHD].astype(f)[:, None]
        w2 = np.asarray(w)[HHD:].astype(f)[:, None]
        return np.ascontiguousarray(
            np.concatenate([cosT * w1, sinT * w2, sinT * w1, cosT * w2], axis=0))

    qrope = rope_pack(q_norm_w)
    krope = rope_pack(k_norm_w)

    trim = np.ascontiguousarray(
        (np.arange(128)[None, :] >= np.arange(128)[:, None]).astype(f))

    indq = np.zeros((128, 8), f)
    indq[0:64, 0] = 1.0     # even chunk -> group rows 0,1
    indq[64:128, 1] = 1.0
    indq[0:64, 6] = 1.0     # odd chunk -> group rows 2,3
    indq[64:128, 7] = 1.0

    ones32 = np.ones((128, 32), f)

    indh = np.zeros((4, 256), f)
    indh[0, 0:64] = 1.0     # even chunk: head row 0 -> partitions 0-63
    indh[1, 64:128] = 1.0
    indh[2, 128 + 0:128 + 64] = 1.0  # odd chunk
    indh[3, 128 + 64:128 + 128] = 1.0

    return xT, wqT, wkvT, wcT, qrope, krope, trim, indq, indh, ones32


def kernel(x, w_q, w_kv, w_c, q_norm_w, k_norm_w):
    xT, wqT, wkvT, wcT, qrope, krope, trim, indq, indh, ones32 = _host_prep(
        x, w_q, w_kv, w_c, q_norm_w, k_norm_w)
    nc = _get_nc()
    in_maps = [
        {"xT": np.ascontiguousarray(xT[b]), "wqT": wqT, "wkvT": wkvT,
         "wcT": wcT, "qrope": qrope, "krope": krope, "trim": trim,
         "indq": indq, "indh": indh, "ones32": ones32}
        for b in range(B)
    ]
    res = run_bass_kernel_spmd(nc, in_maps, list(range(B)))
    y = np.stack([res.results[b]["out"] for b in range(B)], axis=0)
    return y.astype(np.float32)


# revision 26
# speedup vs baseline: 1.2113x; 1.2113x over previous
"""Trainium2 Bass kernel for a GQA attention block (B=8,T=1024,C=1024,H=16,HKV=4).

One batch element per NeuronCore (8 cores). Per core:
  q = x@w_q.T ; kv = x@w_kv.T ; QK-RMSNorm ; RoPE ; GQA attention with
  soft logit cap 50*tanh(s/50), causal softmax ; y = att_out @ w_c.T.

Key design:
  - All matmuls in float32r (full PE rate at N>=256), fp32 PSUM accumulate.
  - Projections in transposed layout [o, t]: per-head tiles are [HD, T] =
    exactly the lhsT/rhs layout QK^T needs. v in natural [t, o] layout with a
    ones column appended so att@V emits softmax denominators for free.
  - Soft cap => no softmax max-subtraction needed (|logit| <= 50).
  - rstd = exp(-0.5*ln(ms/HD + eps)) on ACT (Rsqrt table is banned);
    partition broadcasts on idle GPSIMD; RMSNorm weights folded into
    host-precomputed RoPE tables; rstd folded into q/k prescale.
  - Scores transposed (s^T [kt, qt]) so p^T feeds att@V directly; causal via
    partial-N matmuls + one constant [128,128] upper-triangular mask on
    diagonal blocks; junk in never-streamed PSUM regions is bounded by tanh
    and never read by att@V.
"""

import sys

sys.path.insert(0, "/opt/trn_rl_repo")

import numpy as np

import concourse.bass as bass  # noqa: F401
import concourse.mybir as mybir
from concourse import bacc
from concourse import tile
from concourse.bass_utils import run_bass_kernel_spmd

F32 = mybir.dt.float32
F32R = mybir.dt.float32r
AF = mybir.ActivationFunctionType

B, T, C = 8, 1024, 1024
H, HKV, HD = 16, 4, 64
G = H // HKV          # 4
CAP = 50.0
THETA = 10000.0
EPS = 1e-6
NCH = C // 128        # 8 contraction chunks
QCH = 8               # q output chunks (2 heads each)
KCH = 2               # k output chunks
TT = T // 128         # 8 t subtiles
HHD = HD // 2         # 32


def r(ap):
    """float32r view of an fp32 AP (for matmul inputs)."""
    return ap.bitcast(F32R)


def _build(dbg=False):
    nc = bacc.Bacc("TRN2", target_bir_lowering=False, debug=True)

    xT = nc.dram_tensor("xT", [C, T], F32R, kind="ExternalInput")
    wqT = nc.dram_tensor("wqT", [C, C], F32R, kind="ExternalInput")
    wkvT = nc.dram_tensor("wkvT", [C, 512], F32R, kind="ExternalInput")
    wcT = nc.dram_tensor("wcT", [C, C], F32R, kind="ExternalInput")
    qrope = nc.dram_tensor("qrope", [128, 2 * T], F32, kind="ExternalInput")
    krope = nc.dram_tensor("krope", [128, 2 * T], F32, kind="ExternalInput")
    trim = nc.dram_tensor("trim", [128, 128], F32, kind="ExternalInput")
    indq = nc.dram_tensor("indq", [128, 8], F32R, kind="ExternalInput")
    indh = nc.dram_tensor("indh", [4, 256], F32R, kind="ExternalInput")
    ones32 = nc.dram_tensor("ones32", [128, 32], F32R, kind="ExternalInput")
    den_dram = nc.dram_tensor("den_dram", [1, 32 * 512], F32)
    out = nc.dram_tensor("out", [T, C], F32, kind="ExternalOutput")
    dbgt = {}
    if dbg:
        for name, shape in [("d_qhat", [128, QCH * T]), ("d_khat", [128, KCH * T]),
                            ("d_ksw", [128, KCH * T]), ("d_vhat", [128, TT * HKV * 65]),
                            ("d_pT0", [128, 2048]), ("d_pT1", [128, 4096]),
                            ("d_den", [64, 1024]), ("d_yatt", [128, QCH * T])]:
            dbgt[name] = nc.dram_tensor(name, shape, F32, kind="ExternalOutput")

    with tile.TileContext(nc) as tc:
        with (
            tc.tile_pool(name="const", bufs=1) as const,
            tc.tile_pool(name="big", bufs=1) as big,
            tc.tile_pool(name="wq_pool", bufs=8) as wq_pool,
            tc.tile_pool(name="wc_pool", bufs=8) as wc_pool,
            tc.tile_pool(name="work", bufs=2) as work,
            tc.tile_pool(name="attn", bufs=1) as attn,
            tc.tile_pool(name="psum", bufs=1, space="PSUM") as psum,
        ):
            # ---------------- constants ----------------
            zeros_c = const.tile([128, 1], F32)
            nc.vector.memset(zeros_c, 0.0)
            eps_c = const.tile([128, 1], F32)
            nc.vector.memset(eps_c, EPS)
            nc.const_aps.aps[(F32, 0.0)] = zeros_c
            nc.const_aps.aps[(F32, EPS)] = eps_c

            qrope_sb = const.tile([128, 2 * T], F32)
            nc.sync.dma_start(qrope_sb, qrope[:])
            krope_sb = const.tile([128, 2 * T], F32)
            nc.sync.dma_start(krope_sb, krope[:])
            tri_sb = const.tile([128, 128], F32)
            nc.sync.dma_start(tri_sb, trim[:])
            indq_sb = const.tile([128, 8], F32R)
            nc.sync.dma_start(indq_sb, indq[:])
            indh_sb = const.tile([4, 256], F32R)
            nc.sync.dma_start(indh_sb, indh[:])
            # denominator staging: rows {0,32} = qi {0,1}; columns double-
            # buffered by head parity so adjacent heads don't serialize
            denstg = const.tile([128, 1024], F32)
            nc.vector.memset(denstg, 1.0)

            # ---------------- resident activations ----------------
            xsb = big.tile([128, NCH * T], F32R, tag="xy")  # x^T chunks
            for cc in range(NCH):
                nc.sync.dma_start(xsb[:, cc * T:(cc + 1) * T],
                                  xT[cc * 128:(cc + 1) * 128, :])
            qhat = big.tile([128, QCH * T], F32R, tag="qhat")
            khat = big.tile([128, KCH * T], F32R, tag="khat")
            # partition-swapped copy of khat (PE needs lhsT/rhs at same base)
            khat_sw = big.tile([128, KCH * T], F32R, tag="khat_sw")
            vhat = big.tile([128, TT * (HKV * 65)], F32R, tag="vhat")
            # ones columns (one per (tch, kv-head)) via a single strided DMA
            nc.sync.dma_start(vhat[:, 64:TT * (HKV * 65):65], ones32[:])

            # kv weights: [128, 512] x 8 chunks, in wc_pool's slots (wc loads
            # happen after v-proj is done, so the slots rotate naturally).
            wkv_tiles = []
            for cc in range(NCH):
                wkv_t = wc_pool.tile([128, 512], F32R, tag="wc", name=f"wkv{cc}")
                nc.sync.dma_start(wkv_t, wkvT[cc * 128:(cc + 1) * 128, :])
                wkv_tiles.append(wkv_t)

            # ---------------- transposed projection (+sumsq+RoPE+rstd) ------
            def proj_T(och_total, get_w, rope_sb, hat):
                """och pairs form rstd groups of 4 heads each."""
                mq = {}
                for och in range(och_total):
                    g = och // 2
                    for th in range(2):
                        ps = psum.tile([128, 512], F32, tag="pav", bufs=4,
                                       name=f"pp{och}_{th}")
                        for cc in range(NCH):
                            nc.tensor.matmul(
                                ps,
                                lhsT=r(get_w(cc, och)),
                                rhs=r(xsb[:, cc * T + th * 512:cc * T + (th + 1) * 512]),
                                start=(cc == 0), stop=(cc == NCH - 1),
                            )
                        # raw sumsq over head dims (accumulate over och pair);
                        # ACT Square (one PSUM read; DVE would need two)
                        q2t = work.tile([128, 512], F32R, tag="q2", bufs=2)
                        nc.scalar.square(q2t, ps)
                        if (g, th) not in mq:
                            # shares PSUM slots with the (later-phase) av tiles
                            mq[(g, th)] = psum.tile([4, 512], F32, tag="pav",
                                                    bufs=4, name=f"mq{g}_{th}")
                        ind = indq_sb[:, 0:4] if och % 2 == 0 else indq_sb[:, 4:8]
                        nc.tensor.matmul(mq[(g, th)], lhsT=r(ind), rhs=r(q2t),
                                         start=(och % 2 == 0), stop=(och % 2 == 1))
                        # RoPE via A + B form: A = ps*[cw1;cw2;...],
                        # B = swap32(ps)*[sw2;-sw1;...]  (tables hold weights)
                        hb = och * T + th * 512
                        rsA = slice(th * 512, (th + 1) * 512)
                        rsB = slice(T + th * 512, T + (th + 1) * 512)
                        ta = work.tile([128, 512], F32, tag="ropea", bufs=3)
                        nc.vector.tensor_mul(ta, ps, rope_sb[:, rsA])
                        tb = work.tile([128, 512], F32, tag="ropeb", bufs=3)
                        for blk in range(4):
                            src = (blk ^ 1) * 32
                            nc.vector.tensor_mul(
                                tb[blk * 32:(blk + 1) * 32, :],
                                ps[src:src + 32, :],
                                rope_sb[blk * 32:(blk + 1) * 32, rsB])
                        nc.vector.tensor_add(hat[:, hb:hb + 512], ta, tb)
                    if och % 2 == 1:
                        # rstd for heads 4g..4g+3, then prescale hat rows
                        msq_t = work.tile([4, T], F32, tag="msq_sb", bufs=2)
                        for th in range(2):
                            nc.vector.tensor_copy(
                                msq_t[:, th * 512:(th + 1) * 512], mq[(g, th)])
                        lnt = work.tile([4, T], F32, tag="lnt", bufs=2)
                        nc.scalar.activation(lnt, msq_t, AF.Ln,
                                             bias=EPS, scale=1.0 / HD)
                        rstd_t = work.tile([4, T], F32R, tag="rstd", bufs=2)
                        nc.scalar.activation(rstd_t, lnt, AF.Exp, scale=-0.5)
                        # broadcast rstd across head rows via indicator matmul,
                        # then prescale hat in one [128,512] mul per (och, th)
                        for oo in range(2):
                            oc = g * 2 + oo
                            ind2 = indh_sb[:, oo * 128:(oo + 1) * 128]
                            for th in range(2):
                                bc = psum.tile([128, 512], F32, tag="pav",
                                               bufs=4, name=f"bc{oc}_{th}")
                                nc.tensor.matmul(
                                    bc, lhsT=r(ind2),
                                    rhs=r(rstd_t[:, th * 512:(th + 1) * 512]),
                                    start=True, stop=True)
                                sl = slice(oc * T + th * 512,
                                           oc * T + (th + 1) * 512)
                                nc.vector.tensor_mul(hat[:, sl], hat[:, sl], bc)

            # k projection first (unblocks attention early)
            proj_T(KCH,
                   lambda cc, och: wkv_tiles[cc][:, och * 128:(och + 1) * 128],
                   krope_sb, khat)

            # swapped-half copy of khat for base-partition matching
            for koch in range(KCH):
                sl = slice(koch * T, (koch + 1) * T)
                nc.vector.tensor_copy(khat_sw[0:64, sl], khat[64:128, sl])
                nc.vector.tensor_copy(khat_sw[64:128, sl], khat[0:64, sl])

            # q projection: wq streamed as [128,128] tiles, 8 live per och
            def get_wq(cc, och):
                t_ = wq_pool.tile([128, 128], F32R, tag="wq", name=f"wq{och}_{cc}")
                nc.sync.dma_start(
                    t_, wqT[cc * 128:(cc + 1) * 128, och * 128:(och + 1) * 128])
                return t_

            proj_T(QCH, get_wq, qrope_sb, qhat)

            # v projection LAST among projections (xsb's final reader, so the
            # xsb->yatt slot handoff doesn't stall the attention pipeline)
            for tch in range(TT):
                ps = psum.tile([128, 256], F32, tag="pav", bufs=4,
                               name=f"vps{tch}")
                for cc in range(NCH):
                    nc.tensor.matmul(
                        ps,
                        lhsT=r(xsb[:, cc * T + tch * 128:cc * T + (tch + 1) * 128]),
                        rhs=r(wkv_tiles[cc][:, 256:512]),
                        start=(cc == 0), stop=(cc == NCH - 1),
                    )
                vb = tch * (HKV * 65)
                for n in range(HKV):
                    nc.vector.tensor_copy(vhat[:, vb + n * 65:vb + n * 65 + 64],
                                          ps[:, n * 64:(n + 1) * 64])

            # ---------------- attention ----------------
            yatt = big.tile([128, QCH * T], F32R, tag="xy", name="yatt")
            STACKS = [(0, 0, 0), (1, 0, 1), (1, 4, 1)]  # (qi, j_lo, pT_idx)

            for hp in range(H // 2):
                pair_avs = []
                for hh2 in range(2):
                    h = 2 * hp + hh2
                    och, hh = h // 2, h % 2
                    n = h // G
                    koch, khh = n // 2, n % 2
                    qrow = qhat[hh * 64:(hh + 1) * 64, och * T:(och + 1) * T]
                    ksrc = khat if khh == hh else khat_sw
                    krow = ksrc[hh * 64:(hh + 1) * 64, koch * T:(koch + 1) * T]

                    pT0 = attn.tile([128, 2048], F32R, tag="pT0",
                                    name=f"pT0_{h}")
                    pT1 = attn.tile([128, 4096], F32R, tag="pT1",
                                    name=f"pT1_{h}")
                    pts = [pT0, pT1]

                    for qi, jlo, pti in STACKS:
                        st = psum.tile([128, 2048], F32, tag="stack", bufs=1,
                                       name=f"st{h}_{qi}_{jlo}")
                        for jj in range(4):
                            j = jlo + jj
                            # full N=512 (even below-diagonal): keeps PSUM
                            # fully written; att@V never streams the junk
                            nc.tensor.matmul(
                                st[:, jj * 512:(jj + 1) * 512],
                                lhsT=r(krow[:, j * 128:(j + 1) * 128]),
                                rhs=r(qrow[:, qi * 512:(qi + 1) * 512]),
                                start=True, stop=True,
                            )
                        pcol = (jlo // 4) * 2048 if pti == 1 else 0
                        nc.scalar.activation(pts[pti][:, pcol:pcol + 2048], st,
                                             AF.Tanh, scale=1.0 / (8.0 * CAP))
                    # exp(50*t) in place
                    nc.scalar.activation(pT0, pT0, AF.Exp, scale=CAP)
                    nc.scalar.activation(pT1, pT1, AF.Exp, scale=CAP)
                    # diagonal masks + junk zeroing on straddle stacks
                    for pt, base in ((pT0, 0), (pT1, 2048)):
                        for jj in range(4):
                            col = base + jj * 512 + jj * 128
                            nc.vector.tensor_mul(pt[:, col:col + 128],
                                                 pt[:, col:col + 128], tri_sb)
                        zsl = pt[:, base + 3 * 512 + 256:
                                 base + 3 * 512 + 384]
                        nc.vector.tensor_scalar_mul(zsl, zsl, 0.0)

                    if dbg and h == 0:
                        nc.sync.dma_start(dbgt["d_pT0"][:], pT0.bitcast(F32))
                        nc.sync.dma_start(dbgt["d_pT1"][:], pT1.bitcast(F32))
                    # att@V (+ denominator row via ones column)
                    for qi in range(2):
                        av = psum.tile([65, 512], F32, tag="pav", bufs=4,
                                       name=f"av{h}_{qi}")
                        pair_avs.append((h, qi, av))
                        pt = pts[qi]
                        jhi = 4 * (qi + 1)
                        for j in range(jhi):
                            rr_ = j - 4 * qi
                            off = 0 if rr_ < 0 else min(128 * rr_, 256)
                            nc.tensor.matmul(
                                av[:, off:512],
                                lhsT=r(vhat[:, j * (HKV * 65) + n * 65:
                                            j * (HKV * 65) + (n + 1) * 65]),
                                rhs=r(pt[:, j * 512 + off:(j + 1) * 512]),
                                start=(j == 0), stop=(j == jhi - 1),
                            )
                # one reciprocal for the pair's 4 denominators (rows 0/32/64/96
                # of denstg, col double-buffered by pair parity)
                dcol = (hp % 2) * 512
                for u, (h, qi, av) in enumerate(pair_avs):
                    nc.vector.tensor_copy(
                        denstg[32 * u:32 * u + 1, dcol:dcol + 512],
                        av[64:65, :])
                nc.vector.reciprocal(denstg[0:97, dcol:dcol + 512],
                                     denstg[0:97, dcol:dcol + 512])
                # broadcast via DRAM round-trip (gpsimd partition_broadcast
                # reads the wrong partition on HW for offset sources)
                for u, (h, qi, av) in enumerate(pair_avs):
                    och, hh = h // 2, h % 2
                    dsl = den_dram[0:1, (h * 2 + qi) * 512:
                                   (h * 2 + qi + 1) * 512]
                    nc.sync.dma_start(dsl,
                                      denstg[32 * u:32 * u + 1,
                                             dcol:dcol + 512])
                    rb2 = work.tile([64, 512], F32, tag="rb", bufs=3)
                    bsrc = bass.AP(tensor=dsl.tensor, offset=dsl.offset,
                                   ap=[[0, 64], [1, 512]])
                    nc.sync.dma_start(rb2, bsrc)
                    nc.vector.tensor_mul(
                        yatt[hh * 64:(hh + 1) * 64,
                             och * T + qi * 512:och * T + (qi + 1) * 512],
                        av[0:64, :], rb2)

            if dbg:
                nc.sync.dma_start(dbgt["d_qhat"][:], qhat.bitcast(F32))
                nc.sync.dma_start(dbgt["d_khat"][:], khat.bitcast(F32))
                nc.sync.dma_start(dbgt["d_ksw"][:], khat_sw.bitcast(F32))
                nc.sync.dma_start(dbgt["d_vhat"][:], vhat.bitcast(F32))
                nc.sync.dma_start(dbgt["d_yatt"][:], yatt.bitcast(F32))
            # ---------------- c_proj ----------------
            for oh in range(2):
                wc_tiles = []
                for cc in range(NCH):
                    wc_t = wc_pool.tile([128, 512], F32R, tag="wc",
                                        name=f"wc{oh}_{cc}")
                    nc.sync.dma_start(
                        wc_t, wcT[cc * 128:(cc + 1) * 128,
                                  oh * 512:(oh + 1) * 512])
                    wc_tiles.append(wc_t)
                for tch in range(TT):
                    ps = psum.tile([128, 512], F32, tag="pav", bufs=4,
                                   name=f"cp{oh}_{tch}")
                    for cc in range(NCH):
                        nc.tensor.matmul(
                            ps,
                            lhsT=r(yatt[:, cc * T + tch * 128:
                                        cc * T + (tch + 1) * 128]),
                            rhs=r(wc_tiles[cc]),
                            start=(cc == 0), stop=(cc == NCH - 1),
                        )
                    osb = work.tile([128, 512], F32, tag="osb", bufs=2)
                    nc.vector.tensor_copy(osb, ps)
                    nc.sync.dma_start(
                        out[tch * 128:(tch + 1) * 128, oh * 512:(oh + 1) * 512],
                        osb)

    nc.compile()
    return nc


_NC_CACHE = None


def _get_nc():
    global _NC_CACHE
    if _NC_CACHE is None:
        _NC_CACHE = _build()
    return _NC_CACHE


def _host_prep(x, w_q, w_kv, w_c, q_norm_w, k_norm_w):
    f = np.float32
    xT = np.ascontiguousarray(np.transpose(np.asarray(x), (0, 2, 1))).astype(f, copy=False)
    wqT = np.ascontiguousarray(np.asarray(w_q).T).astype(f, copy=False)
    wkvT = np.ascontiguousarray(np.asarray(w_kv).T).astype(f, copy=False)
    wcT = np.ascontiguousarray(np.asarray(w_c).T).astype(f, copy=False)

    inv_freq = 1.0 / (THETA ** (np.arange(0, HD, 2, dtype=np.float32) / HD))
    pos = np.arange(T, dtype=np.float32)
    freqs = np.outer(pos, inv_freq)            # [T, 32]
    cosT = np.cos(freqs).T.astype(f)           # [32, T]
    sinT = np.sin(freqs).T.astype(f)

    def rope_pack(w):
        w1 = np.asarray(w)[:HHD].astype(f)[:, None]
        w2 = np.asarray(w)[HHD:].astype(f)[:, None]
        ta = np.concatenate([cosT * w1, cosT * w2, cosT * w1, cosT * w2], axis=0)
        tb = np.concatenate([sinT * w2, -sinT * w1, sinT * w2, -sinT * w1],
                            axis=0)
        return np.ascontiguousarray(np.concatenate([ta, tb], axis=1))

    qrope = rope_pack(q_norm_w)
    krope = rope_pack(k_norm_w)

    trim = np.ascontiguousarray(
        (np.arange(128)[None, :] >= np.arange(128)[:, None]).astype(f))

    indq = np.zeros((128, 8), f)
    indq[0:64, 0] = 1.0     # even chunk -> group rows 0,1
    indq[64:128, 1] = 1.0
    indq[0:64, 6] = 1.0     # odd chunk -> group rows 2,3
    indq[64:128, 7] = 1.0

    ones32 = np.ones((128, 32), f)

    indh = np.zeros((4, 256), f)
    indh[0, 0:64] = 1.0     # even chunk: head row 0 -> partitions 0-63
    indh[1, 64:128] = 1.0
    indh[2, 128 + 0:128 + 64] = 1.0  # odd chunk
    indh[3, 128 + 64:128 + 128] = 1.0

    return xT, wqT, wkvT, wcT, qrope, krope, trim, indq, indh, ones32


def kernel(x, w_q, w_kv, w_c, q_norm_w, k_norm_w):
    xT, wqT, wkvT, wcT, qrope, krope, trim, indq, indh, ones32 = _host_prep(
        x, w_q, w_kv, w_c, q_norm_w, k_norm_w)
    nc = _get_nc()
    in_maps = [
        {"xT": np.ascontiguousarray(xT[b]), "wqT": wqT, "wkvT": wkvT,
         "wcT": wcT, "qrope": qrope, "krope": krope, "trim": trim,
         "indq": indq, "indh": indh, "ones32": ones32}
        for b in range(B)
    ]
    res = run_bass_kernel_spmd(nc, in_maps, list(range(B)))
    y = np.stack([res.results[b]["out"] for b in range(B)], axis=0)
    return y.astype(np.float32)
